# revision 28
# baseline (speedup 1.0000x reference)
"""Trainium2 Bass kernel for nn_MultiHeadAttn (dense transformer block).

Contract: kernel(**inputs) takes the FULL unsharded inputs from
reference.setup_inputs() and returns the FULL output [8, 1024, 768] f32.

Sharding: data-parallel over batch N=8 -> one batch item per NeuronCore
(8 cores), no collectives.

Per-core design (channels on partitions, sequence on the free dim; host
does the boundary transposes):
  xT [768,1024] -> per 128-channel tile (2 heads):
    qT dense blockdiag (softmax scale folded), kd dense blockdiag (ONE
    drain), v natural [keys,hid] via a 256-wide moving operand (full-rate
    f32r), per-head scores via 64-partition matmuls on the dense kd/qT
    halves (tile_position rows 0/64), exp on ScalarE (PSUM drain), AV
    with a ones-column in lhsT (M=65) giving per-q exp-rowsums for free.
  Per-tile epilogue (pipelined into the next tile's score loop):
    drain uA/uB, reciprocal the rowsum rows, gpsimd-broadcast them,
    p2 = wpA'@uAs + wpB'@uBs + bpA*rowsumA + bpB*rowsumB (proj bias
    folded as rank-1 matmuls), at = p2 * recip (one DVE op), sq = at*at
    in bf16 for the LN sumsq.
  Tail: LN stats via ones-matmuls, FF on RAW at overlapping the rstd
    chain; LN algebra: normed@W = (a@W - colsum*mu)*rstd, the -colsum*mu
    folded into the FF PSUM accumulation, y = (ff*rstdB + bff) + at.
  All weights arrive in two packed blobs (2 DMAs) to minimize HWDGE
    descriptor-generation serialization at kernel start.

Channel permutation: attention output channels are produced head-major
(c' = h*64+dh) while the module interleaves (c = dh*12+h). LN is
permutation-invariant; wff rows/cols, bff, gamma, beta are permuted on
the host and the final output is unpermuted on the host. bk drops out
exactly: q.(k+bk) = q.k + (q.bk), constant per query across keys, and
softmax is shift-invariant.
"""

import numpy as np

import concourse.bacc as bacc
import concourse.mybir as mybir
import concourse.tile as tile
from concourse.bass_utils import run_bass_kernel_spmd

F32 = mybir.dt.float32
F32R = mybir.dt.float32r
BF16 = mybir.dt.bfloat16
AF = mybir.ActivationFunctionType
OP = mybir.AluOpType

S = 1024  # sequence length
D = 768  # model dim
H = 12  # heads
DH = 64  # head dim
NT = 6  # channel tiles of 128 (2 heads each)
KC = 8  # key chunks of 128
LN_EPS = 1e-5

# packed weight blob column offsets (wpk [128, WPK])
_WQ, _WK, _WV = 0, 128, 256
_WPA, _WPB = 512, 640
_BPA, _BPB = 768, 896
_BQ = 1024  # [128,1]
_NCS = 1025  # [1,768]
_ONE = 1793  # [128,1]
_BFF = 1794  # [128,6]
WPK = 1800

_CACHE = {}


def build_nc(loop_n=None, debug=False):
    """Build the single-core bass program (SPMD across 8 cores)."""
    nc = bacc.Bacc("TRN2", target_bir_lowering=False, debug=False)

    xT_d = nc.dram_tensor("xT", [D, S], F32R, kind="ExternalInput")
    wpk_d = nc.dram_tensor("wpk", [128, WPK], F32R, kind="ExternalInput")
    wffpk_d = nc.dram_tensor("wffpk", [128, NT * D], F32R, kind="ExternalInput")
    dcs_d = nc.dram_tensor("dcs", [128, NT * 128], F32R, kind="ExternalInput")
    out_d = nc.dram_tensor("out", [D, S], F32, kind="ExternalOutput")
    dbg = {}
    if debug:
        for nm, shp in [
            ("dbg_q", [128, S]), ("dbg_k", [128, S]), ("dbg_v", [128, 1040]),
            ("dbg_eA", [128, S]), ("dbg_eB", [128, S]),
            ("dbg_uAs", [65, S]), ("dbg_uBs", [65, S]),
            ("dbg_at", [128, S]), ("dbg_mean", [1, S]), ("dbg_rstd", [1, S]),
        ]:
            dbg[nm] = nc.dram_tensor(nm, shp, F32, kind="ExternalOutput")

    with tile.TileContext(nc) as tc:

        def body(_i=None):
            with (
                tc.tile_pool(name="const", bufs=1) as cpool,
                tc.tile_pool(name="atile", bufs=1) as apool,
                tc.tile_pool(name="psS", bufs=2, space="PSUM") as psS,
                tc.tile_pool(name="psU", bufs=2, space="PSUM") as psU,
                tc.tile_pool(name="qkv", bufs=2) as qkvpool,
                tc.tile_pool(name="xr", bufs=1) as xrpool,
                tc.tile_pool(name="ew", bufs=2) as ew,
                tc.tile_pool(name="uw", bufs=2) as uw,
                tc.tile_pool(name="w3", bufs=2) as w3,
                tc.tile_pool(name="s3", bufs=1) as s3,
            ):
                # xr(0) first: the DMA engine serializes transfers, and the
                # first q-proj only needs xr+wpk. wffpk (biggest, tail-only)
                # goes last on a separate queue.
                xr0 = xrpool.tile([128, S], F32R, name="xr", tag="xr", bufs=2)
                nc.sync.dma_start(xr0[:], xT_d[0:128, :])
                wpk = cpool.tile([128, WPK], F32R, name="wpk")
                nc.sync.dma_start(wpk[:], wpk_d[:])
                wffpk2 = cpool.tile([128, NT * 128], F32R, name="wffpk2")
                nc.sync.dma_start(wffpk2[:], dcs_d[:])
                wffpk = cpool.tile([128, NT * D], F32R, name="wffpk")

                wq2r = wpk[:, _WQ : _WQ + 128]
                wk2r = wpk[:, _WK : _WK + 128]
                wv256r = wpk[:, _WV : _WV + 256]
                wp2r = wpk[:, _WPA : _WPA + 128]
                bp2c = wpk[:, _BPA : _BPA + 1].bitcast(F32)
                bq2 = wpk[:, _BQ : _BQ + 1].bitcast(F32)
                dcs_m = [
                    wffpk2[:, 128 * m : 128 * (m + 1)] for m in range(NT)
                ]
                onesr = wpk[:, _ONE : _ONE + 1]
                gb = [
                    wpk[:, _BFF + t : _BFF + t + 1].bitcast(F32)
                    for t in range(NT)
                ]
                wffr = [wffpk[:, D * t : D * (t + 1)] for t in range(NT)]
                onesb = cpool.tile([128, 1], BF16, name="onesb")
                nc.vector.tensor_copy(onesb[:], onesr[:].bitcast(F32))

                qT = [None] * NT
                kd = [None] * NT
                vs = [None] * NT
                uAp = [None] * NT
                uBp = [None] * NT
                aT = [None] * NT
                sqs = [None] * NT

                xrs = {0: xr0}

                def fetch_x(t):
                    xr = xrpool.tile([128, S], F32R, name="xr", tag="xr", bufs=2)
                    nc.sync.dma_start(xr[:], xT_d[128 * t : 128 * (t + 1), :])
                    xrs[t] = xr

                def proj_pair(t):
                    """q/k/v projections for channel tile t (v first: its
                    PSUM slots must free fastest for the score rotation)."""
                    xr = xrs[t]
                    qps = psS.tile([128, S], F32, name="qps", tag="s")
                    for qh in range(2):
                        nc.tensor.matmul(
                            qps[:, 512 * qh : 512 * (qh + 1)],
                            wq2r,
                            xr[:, 512 * qh : 512 * (qh + 1)],
                            start=True,
                            stop=True,
                        )
                    qa = qkvpool.tile([128, S], BF16, name=f"qTA{t}", tag="qa")
                    nc.vector.tensor_scalar_add(qa[0:64, :], qps[0:64, :], bq2[0:64])
                    qb = qkvpool.tile([128, S], BF16, name=f"qTB{t}", tag="qb")
                    nc.vector.tensor_scalar_add(
                        qb[64:128, :], qps[64:128, :], bq2[64:128]
                    )
                    if t < 2:  # later tiles reuse the slots; pads stay zero
                        U16 = mybir.dt.uint16
                        nc.vector.memset(qa[64:128, :].bitcast(U16), 0)
                        nc.vector.memset(qb[0:64, :].bitcast(U16), 0)
                    qT[t] = (qa, qb)
                    kps = psS.tile([128, S], F32, name="kps", tag="s")
                    for qh in range(2):
                        nc.tensor.matmul(
                            kps[:, 512 * qh : 512 * (qh + 1)],
                            wk2r,
                            xr[:, 512 * qh : 512 * (qh + 1)],
                            start=True,
                            stop=True,
                        )
                    k = qkvpool.tile([128, S], BF16, name=f"kd{t}", tag="k")
                    nc.vector.tensor_copy(k[:], kps[:])
                    kd[t] = k
                    v = qkvpool.tile([128, KC, 130], BF16, name=f"vs{t}", tag="v")
                    for g in range(2):
                        vps = psS.tile([128, 4, 256], F32, name="vps", tag="s")
                        for j in range(4):
                            c = 4 * g + j
                            nc.tensor.matmul(
                                vps[:, j, :],
                                xr[:, 128 * c : 128 * (c + 1)],
                                wv256r,
                                start=True,
                                stop=True,
                            )
                        nc.vector.tensor_copy(
                            v[:, 4 * g : 4 * (g + 1), 1:65], vps[:, :, 0:64]
                        )
                        nc.vector.tensor_copy(
                            v[:, 4 * g : 4 * (g + 1), 66:130], vps[:, :, 64:128]
                        )
                    nc.vector.memset(
                        v[:, :, 0:1].bitcast(mybir.dt.uint16), 0x3F80
                    )
                    nc.vector.memset(
                        v[:, :, 65:66].bitcast(mybir.dt.uint16), 0x3F80
                    )
                    vs[t] = v
                    if debug and t == 0:
                        pass
                        pass
                        pass
                        pass

                def drain_u(t):
                    """Drain AV accumulators; reciprocal + broadcast the
                    softmax denominators (issued at tile boundary, runs on
                    DVE/Pool off the PE critical path)."""
                    uAs = uw.tile([65, S], F32R, name="uAs", tag="uAs")
                    nc.vector.tensor_copy(uAs[:], uAp[t][:])
                    uBs = uw.tile([65, S], F32R, name="uBs", tag="uBs")
                    nc.vector.tensor_copy(uBs[:], uBp[t][:])
                    if debug and t == 0:
                        nc.sync.dma_start(dbg["dbg_uAs"][:], uAs[:].bitcast(F32))
                        nc.sync.dma_start(dbg["dbg_uBs"][:], uBs[:].bitcast(F32))
                    rcA = uw.tile([1, S], F32, name="rcA", tag="rcA", bufs=1)
                    nc.vector.reciprocal(rcA[:], uAs[0:1, :].bitcast(F32))
                    rcB = uw.tile([1, S], F32, name="rcB", tag="rcB", bufs=1)
                    nc.vector.reciprocal(rcB[:], uBs[0:1, :].bitcast(F32))
                    rb2 = uw.tile([128, S], F32, name="rb2", tag="rb2", bufs=1)
                    nc.gpsimd.partition_broadcast(rb2[0:64, :], rcA[:])
                    # partition_broadcast ignores the out AP's partition base,
                    # so broadcast B at base 0 and DMA-shift it up.
                    rbB0 = uw.tile([64, S], F32, name="rbB0", tag="rbB0", bufs=1)
                    nc.gpsimd.partition_broadcast(rbB0[:], rcB[:])
                    nc.sync.dma_start(rb2[64:128, :], rbB0[:])
                    u2 = uw.tile([128, S], F32R, name="u2", tag="u2", bufs=1)
                    nc.sync.dma_start(u2[0:64, :], uAs[1:65, :])
                    nc.sync.dma_start(u2[64:128, :], uBs[1:65, :])
                    return uAs, uBs, rb2, u2

                def epilogue(t, uAs, uBs, rb2, u2):
                    """proj + softmax-normalize for tile t (issued at kc==2
                    of tile t+1 so its PSUM slot doesn't stall scores)."""
                    p2 = psS.tile([128, S], F32, name="p2", tag="s")
                    for qh in range(2):
                        nc.tensor.matmul(
                            p2[:, 512 * qh : 512 * (qh + 1)],
                            wp2r,
                            u2[:, 512 * qh : 512 * (qh + 1)],
                            start=True,
                            stop=True,
                        )
                    a1 = uw.tile([128, S], F32, name="a1", tag="a1", bufs=1)
                    nc.vector.tensor_mul(a1[:], p2[:], rb2[:])
                    at = apool.tile([128, S], F32R, name=f"aT{t}")
                    nc.vector.tensor_scalar_add(at[:], a1[:], bp2c)
                    aT[t] = at
                    if debug and t == 0:
                        nc.sync.dma_start(dbg["dbg_at"][:], at[:].bitcast(F32))
                    sq = apool.tile([128, S], BF16, name=f"sq{t}")
                    nc.vector.tensor_mul(sq[:], at[:], at[:])
                    sqs[t] = sq

                # ---- main attention loop ----
                proj_pair(0)
                pend = {}
                for t in range(NT):
                    uA = psU.tile([65, S], F32, name="uA", tag="u")
                    uB = psU.tile([65, S], F32, name="uB", tag="u")
                    uAp[t] = uA
                    uBp[t] = uB
                    es = {}
                    for kc in range(KC):
                        sA = psS.tile([128, S], F32, name="sA", tag="s")
                        sB = psS.tile([128, S], F32, name="sB", tag="s")
                        for qh in range(2):
                            nc.tensor.matmul(
                                sA[:, 512 * qh : 512 * (qh + 1)],
                                kd[t][:, 128 * kc : 128 * (kc + 1)],
                                qT[t][0][:, 512 * qh : 512 * (qh + 1)],
                                start=True,
                                stop=True,
                            )
                        for qh in range(2):
                            nc.tensor.matmul(
                                sB[:, 512 * qh : 512 * (qh + 1)],
                                kd[t][:, 128 * kc : 128 * (kc + 1)],
                                qT[t][1][:, 512 * qh : 512 * (qh + 1)],
                                start=True,
                                stop=True,
                            )
                        eA = ew.tile([128, S], BF16, name="eA", tag="eA", bufs=3)
                        if kc == 2:
                            # Schraudolph exp on DVE: bf16 bits = int16 of
                            # x*128/ln2 + (16256-8.25); offloads ScalarE
                            nc.vector.tensor_scalar(
                                eA[:].bitcast(mybir.dt.int16), sA[:],
                                184.6650292, 16247.75,
                                op0=OP.mult, op1=OP.add,
                            )
                        else:
                            nc.scalar.activation(eA[:], sA[:], AF.Exp)
                        eB = ew.tile([128, S], BF16, name="eB", tag="eB", bufs=3)
                        if kc == 5:
                            nc.vector.tensor_scalar(
                                eB[:].bitcast(mybir.dt.int16), sB[:],
                                184.6650292, 16247.75,
                                op0=OP.mult, op1=OP.add,
                            )
                        else:
                            nc.scalar.activation(eB[:], sB[:], AF.Exp)
                        es[kc] = (eA, eB)
                        if debug and t == 0 and kc == 0:
                            pass
                            pass

                        def av(j):
                            st = j == 0
                            fin = j == KC - 1
                            ea, eb = es[j]
                            for qh in range(2):
                                nc.tensor.matmul(
                                    uA[:, 512 * qh : 512 * (qh + 1)],
                                    vs[t][:, j, 0:65],
                                    ea[:, 512 * qh : 512 * (qh + 1)],
                                    start=st,
                                    stop=fin,
                                )
                            for qh in range(2):
                                nc.tensor.matmul(
                                    uB[:, 512 * qh : 512 * (qh + 1)],
                                    vs[t][:, j, 65:130],
                                    eb[:, 512 * qh : 512 * (qh + 1)],
                                    start=st,
                                    stop=fin,
                                )

                        # AV(kc0) deferred to kc1: its first write to the uA/uB
                        # slots must not stall PE on the previous tile's drains
                        if kc == 0:
                            if t + 1 < NT:
                                fetch_x(t + 1)
                        else:
                            if kc == 1:
                                av(0)
                            av(kc)
                        if kc == 1 and t + 1 < NT:
                            proj_pair(t + 1)
                        if kc == 2:
                            if t >= 1:
                                epilogue(t - 1, *pend[t - 1])
                            if t == 0:
                                nc.sync.dma_start(wffpk[:], wffpk_d[:])
                    pend[t] = drain_u(t)

                # ---- tail: epilogue(5) + LN stats + FF, overlapped ----
                sums = psU.tile([1, S], F32, name="sums", tag="u")
                sumsq = psU.tile([1, S], F32, name="sumsq", tag="u")

                def stats(t):
                    st = t == 0
                    fin = t == NT - 1
                    for qh in range(2):
                        nc.tensor.matmul(
                            sums[:, 512 * qh : 512 * (qh + 1)],
                            onesr,
                            aT[t][:, 512 * qh : 512 * (qh + 1)],
                            start=st,
                            stop=fin,
                        )
                        nc.tensor.matmul(
                            sumsq[:, 512 * qh : 512 * (qh + 1)],
                            onesb[:],
                            sqs[t][:, 512 * qh : 512 * (qh + 1)],
                            start=st,
                            stop=fin,
                        )

                epilogue(NT - 1, *pend[NT - 1])
                for t in range(NT):
                    stats(t)
                mean = s3.tile([1, S], F32R, name="mean")
                nc.vector.tensor_scalar_mul(mean[:], sums[:], 1.0 / D)
                tmpa = s3.tile([1, S], F32, name="tmpa")
                nc.vector.tensor_scalar_mul(tmpa[:], sumsq[:], 1.0 / D)
                tmpb = s3.tile([1, S], F32, name="tmpb")
                nc.vector.tensor_mul(
                    tmpb[:], mean[:].bitcast(F32), mean[:].bitcast(F32)
                )
                nc.vector.scalar_tensor_tensor(
                    tmpa[:], tmpa[:], LN_EPS, tmpb[:], op0=OP.add, op1=OP.subtract
                )
                nc.scalar.sqrt(tmpb[:], tmpa[:])
                rstd = s3.tile([1, S], F32, name="rstd")
                nc.vector.reciprocal(rstd[:], tmpb[:])
                rstdB = s3.tile([128, S], F32, name="rstdB")
                nc.gpsimd.partition_broadcast(rstdB[:], rstd[:])
                meanB = s3.tile([128, S], F32R, name="meanB")
                nc.gpsimd.partition_broadcast(meanB[:], mean[:])
                if debug:
                    nc.sync.dma_start(dbg["dbg_mean"][:], mean[:].bitcast(F32))
                    nc.sync.dma_start(dbg["dbg_rstd"][:], rstd[:])

                ffs = []
                for m in range(NT):
                    ff = psS.tile([128, S], F32, name="ff", tag="s")
                    for kc in range(NT):
                        st = kc == 0
                        for qh in range(2):
                            nc.tensor.matmul(
                                ff[:, 512 * qh : 512 * (qh + 1)],
                                wffr[kc][:, 128 * m : 128 * (m + 1)],
                                aT[kc][:, 512 * qh : 512 * (qh + 1)],
                                start=st,
                                stop=False,
                            )
                    # LN fold: ff -= colsum_m * mean_j via diag(-colsum)
                    for qh in range(2):
                        nc.tensor.matmul(
                            ff[:, 512 * qh : 512 * (qh + 1)],
                            dcs_m[m],
                            meanB[:, 512 * qh : 512 * (qh + 1)],
                            start=False,
                            stop=True,
                        )
                    t1 = w3.tile([128, S], F32, name="t1", tag="t1")
                    nc.vector.tensor_mul(t1[:], ff[:], rstdB[:])
                    y = w3.tile([128, S], F32, name="y", tag="y")
                    nc.vector.scalar_tensor_tensor(
                        y[:], t1[:], gb[m], aT[m][:], op0=OP.add, op1=OP.add
                    )
                    nc.sync.dma_start(out_d[128 * m : 128 * (m + 1), :], y[:])

        if loop_n is not None:
            with tc.For_i(0, loop_n, 1) as i:
                body(i)
        else:
            body()

    nc.compile()
    return nc


def prep_inputs(x, wq, bq, wk, bk, wv, bv, wp, bp, gamma, beta, wff, bff):
    """Host-side preprocessing -> per-core input maps."""
    x = np.asarray(x, dtype=np.float32)
    wq = np.asarray(wq, np.float32)
    bq = np.asarray(bq, np.float32)
    wk = np.asarray(wk, np.float32)
    wv = np.asarray(wv, np.float32)
    wp_ = np.asarray(wp, np.float32)
    bp = np.asarray(bp, np.float32)
    bv = np.asarray(bv, np.float32)
    gamma = np.asarray(gamma, np.float32)
    beta = np.asarray(beta, np.float32)
    wff = np.asarray(wff, np.float32)
    bff = np.asarray(bff, np.float32)

    scale = np.float32(1.0 / np.sqrt(np.float32(DH)))
    wpk = np.zeros((128, WPK), np.float32)
    wpk[0:64, _WQ : _WQ + 64] = wq * scale
    wpk[64:128, _WQ + 64 : _WQ + 128] = wq * scale
    wpk[0:64, _WK : _WK + 64] = wk
    wpk[64:128, _WK + 64 : _WK + 128] = wk
    wpk[0:64, _WV : _WV + 64] = wv
    wpk[64:128, _WV + 64 : _WV + 128] = wv
    wpk[0:64, _WPA : _WPA + 64] = wp_
    wpk[64:128, _WPA + 64 : _WPA + 128] = wp_
    bpp = bv @ wp_ + bp  # v-bias folded through proj
    wpk[:, _BPA] = np.concatenate([bpp, bpp])
    wpk[:, _BQ] = np.concatenate([bq, bq]) * scale

    # channel permutation: head-major c' = h*64+dh holds original c = dh*12+h
    cp = np.arange(D)
    hh, dd = cp // 64, cp % 64
    p = dd * H + hh  # p[c'] = original channel
    wffg = wff * gamma[:, None]  # fold LN gamma into FF rows
    bffg = bff + beta @ wff  # fold LN beta through FF
    wffp = np.ascontiguousarray(wffg[p][:, p]).astype(np.float32)
    ncs = -wffp.sum(axis=0)
    dcs = np.zeros((128, NT * 128), np.float32)
    for m in range(NT):
        np.fill_diagonal(dcs[:, 128 * m : 128 * (m + 1)], ncs[128 * m : 128 * (m + 1)])
    wpk[:, _ONE] = 1.0
    wpk[:, _BFF : _BFF + NT] = bffg[p].reshape(NT, 128).T
    wffpk = np.ascontiguousarray(
        wffp.reshape(NT, 128, D).transpose(1, 0, 2).reshape(128, NT * D)
    )

    shared = {"wpk": wpk, "wffpk": wffpk, "dcs": dcs}
    in_maps = []
    for i in range(x.shape[0]):
        m = dict(shared)
        m["xT"] = np.ascontiguousarray(x[i].T)
        in_maps.append(m)
    return in_maps, p


def postprocess(results, p):
    outs = []
    for r in results:
        yt = r["out"].T  # [S, D] head-major channels
        y = np.empty_like(yt)
        y[:, p] = yt
        outs.append(y)
    return np.stack(outs)


def kernel(**inputs) -> np.ndarray:
    if "nc" not in _CACHE:
        _CACHE["nc"] = build_nc()
    nc = _CACHE["nc"]
    in_maps, p = prep_inputs(**inputs)
    res = run_bass_kernel_spmd(nc, in_maps, list(range(8)))
    return postprocess(res.results, p)


# revision 29
# speedup vs baseline: 1.0368x; 1.0368x over previous
"""Trainium2 Bass kernel for nn_MultiHeadAttn (dense transformer block).

Contract: kernel(**inputs) takes the FULL unsharded inputs from
reference.setup_inputs() and returns the FULL output [8, 1024, 768] f32.

Sharding: data-parallel over batch N=8 -> one batch item per NeuronCore
(8 cores), no collectives.

Per-core design (channels on partitions, sequence on the free dim; host
does the boundary transposes):
  xT [768,1024] -> per 128-channel tile (2 heads):
    qT dense blockdiag (softmax scale folded), kd dense blockdiag (ONE
    drain), v natural [keys,hid] via a 256-wide moving operand (full-rate
    f32r), per-head scores via 64-partition matmuls on the dense kd/qT
    halves (tile_position rows 0/64), exp on ScalarE (PSUM drain), AV
    with a ones-column in lhsT (M=65) giving per-q exp-rowsums for free.
  Per-tile epilogue (pipelined into the next tile's score loop):
    drain uA/uB, reciprocal the rowsum rows, gpsimd-broadcast them,
    p2 = wpA'@uAs + wpB'@uBs + bpA*rowsumA + bpB*rowsumB (proj bias
    folded as rank-1 matmuls), at = p2 * recip (one DVE op), sq = at*at
    in bf16 for the LN sumsq.
  Tail: LN stats via ones-matmuls, FF on RAW at overlapping the rstd
    chain; LN algebra: normed@W = (a@W - colsum*mu)*rstd, the -colsum*mu
    folded into the FF PSUM accumulation, y = (ff*rstdB + bff) + at.
  All weights arrive in two packed blobs (2 DMAs) to minimize HWDGE
    descriptor-generation serialization at kernel start.

Channel permutation: attention output channels are produced head-major
(c' = h*64+dh) while the module interleaves (c = dh*12+h). LN is
permutation-invariant; wff rows/cols, bff, gamma, beta are permuted on
the host and the final output is unpermuted on the host. bk drops out
exactly: q.(k+bk) = q.k + (q.bk), constant per query across keys, and
softmax is shift-invariant.
"""

import numpy as np

import concourse.bacc as bacc
import concourse.mybir as mybir
import concourse.tile as tile
from concourse.bass_utils import run_bass_kernel_spmd

F32 = mybir.dt.float32
F32R = mybir.dt.float32r
BF16 = mybir.dt.bfloat16
AF = mybir.ActivationFunctionType
OP = mybir.AluOpType

S = 1024  # sequence length
D = 768  # model dim
H = 12  # heads
DH = 64  # head dim
NT = 6  # channel tiles of 128 (2 heads each)
KC = 8  # key chunks of 128
LN_EPS = 1e-5

# packed weight blob column offsets (wpk [128, WPK])
_WQ, _WK, _WV = 0, 128, 256
_WPA, _WPB = 512, 640
_BPA, _BPB = 768, 896
_BQ = 1024  # [128,1]
_NCS = 1025  # [1,768]
_ONE = 1793  # [128,1]
_BFF = 1794  # [128,6]
WPK = 1800

_CACHE = {}


def build_nc(loop_n=None, debug=False):
    """Build the single-core bass program (SPMD across 8 cores)."""
    nc = bacc.Bacc("TRN2", target_bir_lowering=False, debug=False)

    xT_d = nc.dram_tensor("xT", [D, S], F32R, kind="ExternalInput")
    wpk_d = nc.dram_tensor("wpk", [128, WPK], F32R, kind="ExternalInput")
    wffpk_d = nc.dram_tensor("wffpk", [128, NT * D], F32R, kind="ExternalInput")
    dcs_d = nc.dram_tensor("dcs", [128, NT * 128], F32R, kind="ExternalInput")
    wqkvf_d = nc.dram_tensor("wqkvf", [128, 513], F32R, kind="ExternalInput")
    out_d = nc.dram_tensor("out", [D, S], F32, kind="ExternalOutput")
    dbg = {}
    if debug:
        for nm, shp in [
            ("dbg_q", [128, S]), ("dbg_k", [128, S]), ("dbg_v", [128, 1040]),
            ("dbg_eA", [128, S]), ("dbg_eB", [128, S]),
            ("dbg_uAs", [65, S]), ("dbg_uBs", [65, S]),
            ("dbg_at", [128, S]), ("dbg_mean", [1, S]), ("dbg_rstd", [1, S]),
        ]:
            dbg[nm] = nc.dram_tensor(nm, shp, F32, kind="ExternalOutput")

    with tile.TileContext(nc) as tc:

        def body(_i=None):
            with (
                tc.tile_pool(name="const", bufs=1) as cpool,
                tc.tile_pool(name="atile", bufs=1) as apool,
                tc.tile_pool(name="psS", bufs=2, space="PSUM") as psS,
                tc.tile_pool(name="psU", bufs=2, space="PSUM") as psU,
                tc.tile_pool(name="qkv", bufs=2) as qkvpool,
                tc.tile_pool(name="xr", bufs=1) as xrpool,
                tc.tile_pool(name="ew", bufs=2) as ew,
                tc.tile_pool(name="uw", bufs=2) as uw,
                tc.tile_pool(name="w3", bufs=2) as w3,
                tc.tile_pool(name="s3", bufs=1) as s3,
            ):
                # xr(0) first: the DMA engine serializes transfers, and the
                # first q-proj only needs xr+wpk. wffpk (biggest, tail-only)
                # goes last on a separate queue.
                xr0 = xrpool.tile([128, S], F32R, name="xr", tag="xr", bufs=2)
                nc.sync.dma_start(xr0[:], xT_d[0:128, :])
                wqkvf = cpool.tile([128, 513], F32R, name="wqkvf")
                nc.sync.dma_start(wqkvf[:], wqkvf_d[:])
                wpk = cpool.tile([128, WPK], F32R, name="wpk")
                nc.sync.dma_start(wpk[:], wpk_d[:])
                wffpk2 = cpool.tile([128, NT * 128], F32R, name="wffpk2")
                nc.sync.dma_start(wffpk2[:], dcs_d[:])
                wffpk = cpool.tile([128, NT * D], F32R, name="wffpk")

                wq2r = wqkvf[:, 0:128]
                wk2r = wqkvf[:, 128:256]
                wv256r = wqkvf[:, 256:512]
                wp2r = wpk[:, _WPA : _WPA + 128]
                bp2c = wpk[:, _BPA : _BPA + 1].bitcast(F32)
                bq2 = wqkvf[:, 512:513].bitcast(F32)
                dcs_m = [
                    wffpk2[:, 128 * m : 128 * (m + 1)] for m in range(NT)
                ]
                onesr = wpk[:, _ONE : _ONE + 1]
                gb = [
                    wpk[:, _BFF + t : _BFF + t + 1].bitcast(F32)
                    for t in range(NT)
                ]
                wffr = [wffpk[:, D * t : D * (t + 1)] for t in range(NT)]
                onesb = cpool.tile([128, 1], BF16, name="onesb")
                nc.vector.tensor_copy(onesb[:], onesr[:].bitcast(F32))

                qT = [None] * NT
                kd = [None] * NT
                vs = [None] * NT
                uAp = [None] * NT
                uBp = [None] * NT
                aT = [None] * NT
                sqs = [None] * NT

                xrs = {0: xr0}

                def fetch_x(t):
                    xr = xrpool.tile([128, S], F32R, name="xr", tag="xr", bufs=2)
                    nc.sync.dma_start(xr[:], xT_d[128 * t : 128 * (t + 1), :])
                    xrs[t] = xr

                def proj_pair(t):
                    """q/k/v projections for channel tile t (v first: its
                    PSUM slots must free fastest for the score rotation)."""
                    xr = xrs[t]
                    qps = psS.tile([128, S], F32, name="qps", tag="s")
                    for qh in range(2):
                        nc.tensor.matmul(
                            qps[:, 512 * qh : 512 * (qh + 1)],
                            wq2r,
                            xr[:, 512 * qh : 512 * (qh + 1)],
                            start=True,
                            stop=True,
                        )
                    qa = qkvpool.tile([128, S], BF16, name=f"qTA{t}", tag="qa")
                    nc.vector.tensor_scalar_add(qa[0:64, :], qps[0:64, :], bq2[0:64])
                    qb = qkvpool.tile([128, S], BF16, name=f"qTB{t}", tag="qb")
                    nc.vector.tensor_scalar_add(
                        qb[64:128, :], qps[64:128, :], bq2[64:128]
                    )
                    if t < 2:  # later tiles reuse the slots; pads stay zero
                        U16 = mybir.dt.uint16
                        nc.vector.memset(qa[64:128, :].bitcast(U16), 0)
                        nc.vector.memset(qb[0:64, :].bitcast(U16), 0)
                    qT[t] = (qa, qb)
                    kps = psS.tile([128, S], F32, name="kps", tag="s")
                    for qh in range(2):
                        nc.tensor.matmul(
                            kps[:, 512 * qh : 512 * (qh + 1)],
                            wk2r,
                            xr[:, 512 * qh : 512 * (qh + 1)],
                            start=True,
                            stop=True,
                        )
                    k = qkvpool.tile([128, S], BF16, name=f"kd{t}", tag="k")
                    nc.vector.tensor_copy(k[:], kps[:])
                    kd[t] = k
                    v = qkvpool.tile([128, KC, 130], BF16, name=f"vs{t}", tag="v")
                    for g in range(2):
                        vps = psS.tile([128, 4, 256], F32, name="vps", tag="s")
                        for j in range(4):
                            c = 4 * g + j
                            nc.tensor.matmul(
                                vps[:, j, :],
                                xr[:, 128 * c : 128 * (c + 1)],
                                wv256r,
                                start=True,
                                stop=True,
                            )
                        nc.vector.tensor_copy(
                            v[:, 4 * g : 4 * (g + 1), 1:65], vps[:, :, 0:64]
                        )
                        nc.vector.tensor_copy(
                            v[:, 4 * g : 4 * (g + 1), 66:130], vps[:, :, 64:128]
                        )
                    nc.vector.memset(
                        v[:, :, 0:1].bitcast(mybir.dt.uint16), 0x3F80
                    )
                    nc.vector.memset(
                        v[:, :, 65:66].bitcast(mybir.dt.uint16), 0x3F80
                    )
                    vs[t] = v
                    if debug and t == 0:
                        pass
                        pass
                        pass
                        pass

                def drain_u(t):
                    """Drain AV accumulators; reciprocal + broadcast the
                    softmax denominators (issued at tile boundary, runs on
                    DVE/Pool off the PE critical path)."""
                    uAs = uw.tile([65, S], F32R, name="uAs", tag="uAs")
                    nc.vector.tensor_copy(uAs[:], uAp[t][:])
                    uBs = uw.tile([65, S], F32R, name="uBs", tag="uBs")
                    nc.vector.tensor_copy(uBs[:], uBp[t][:])
                    if debug and t == 0:
                        nc.sync.dma_start(dbg["dbg_uAs"][:], uAs[:].bitcast(F32))
                        nc.sync.dma_start(dbg["dbg_uBs"][:], uBs[:].bitcast(F32))
                    rcA = uw.tile([1, S], F32, name="rcA", tag="rcA", bufs=1)
                    nc.vector.reciprocal(rcA[:], uAs[0:1, :].bitcast(F32))
                    rcB = uw.tile([1, S], F32, name="rcB", tag="rcB", bufs=1)
                    nc.vector.reciprocal(rcB[:], uBs[0:1, :].bitcast(F32))
                    rb2 = uw.tile([128, S], F32, name="rb2", tag="rb2", bufs=1)
                    nc.gpsimd.partition_broadcast(rb2[0:64, :], rcA[:])
                    # partition_broadcast ignores the out AP's partition base,
                    # so broadcast B at base 0 and DMA-shift it up.
                    rbB0 = uw.tile([64, S], F32, name="rbB0", tag="rbB0", bufs=1)
                    nc.gpsimd.partition_broadcast(rbB0[:], rcB[:])
                    nc.sync.dma_start(rb2[64:128, :], rbB0[:])
                    u2 = uw.tile([128, S], F32R, name="u2", tag="u2", bufs=1)
                    nc.sync.dma_start(u2[0:64, :], uAs[1:65, :])
                    nc.sync.dma_start(u2[64:128, :], uBs[1:65, :])
                    return uAs, uBs, rb2, u2

                def epilogue(t, uAs, uBs, rb2, u2):
                    """proj + softmax-normalize for tile t (issued at kc==2
                    of tile t+1 so its PSUM slot doesn't stall scores)."""
                    p2 = psS.tile([128, S], F32, name="p2", tag="s")
                    for qh in range(2):
                        nc.tensor.matmul(
                            p2[:, 512 * qh : 512 * (qh + 1)],
                            wp2r,
                            u2[:, 512 * qh : 512 * (qh + 1)],
                            start=True,
                            stop=True,
                        )
                    a1 = uw.tile([128, S], F32, name="a1", tag="a1", bufs=1)
                    nc.vector.tensor_mul(a1[:], p2[:], rb2[:])
                    at = apool.tile([128, S], F32R, name=f"aT{t}")
                    nc.vector.tensor_scalar_add(at[:], a1[:], bp2c)
                    aT[t] = at
                    if debug and t == 0:
                        nc.sync.dma_start(dbg["dbg_at"][:], at[:].bitcast(F32))
                    sq = apool.tile([128, S], BF16, name=f"sq{t}")
                    nc.vector.tensor_mul(sq[:], at[:], at[:])
                    sqs[t] = sq

                # ---- main attention loop ----
                proj_pair(0)
                pend = {}
                for t in range(NT):
                    uA = psU.tile([65, S], F32, name="uA", tag="u")
                    uB = psU.tile([65, S], F32, name="uB", tag="u")
                    uAp[t] = uA
                    uBp[t] = uB
                    es = {}
                    for kc in range(KC):
                        sA = psS.tile([128, S], F32, name="sA", tag="s")
                        sB = psS.tile([128, S], F32, name="sB", tag="s")
                        for qh in range(2):
                            nc.tensor.matmul(
                                sA[:, 512 * qh : 512 * (qh + 1)],
                                kd[t][:, 128 * kc : 128 * (kc + 1)],
                                qT[t][0][:, 512 * qh : 512 * (qh + 1)],
                                start=True,
                                stop=True,
                            )
                        for qh in range(2):
                            nc.tensor.matmul(
                                sB[:, 512 * qh : 512 * (qh + 1)],
                                kd[t][:, 128 * kc : 128 * (kc + 1)],
                                qT[t][1][:, 512 * qh : 512 * (qh + 1)],
                                start=True,
                                stop=True,
                            )
                        eA = ew.tile([128, S], BF16, name="eA", tag="eA", bufs=3)
                        nc.scalar.activation(eA[:], sA[:], AF.Exp)
                        eB = ew.tile([128, S], BF16, name="eB", tag="eB", bufs=3)
                        nc.scalar.activation(eB[:], sB[:], AF.Exp)
                        es[kc] = (eA, eB)
                        if debug and t == 0 and kc == 0:
                            pass
                            pass

                        def av(j):
                            st = j == 0
                            fin = j == KC - 1
                            ea, eb = es[j]
                            for qh in range(2):
                                nc.tensor.matmul(
                                    uA[:, 512 * qh : 512 * (qh + 1)],
                                    vs[t][:, j, 0:65],
                                    ea[:, 512 * qh : 512 * (qh + 1)],
                                    start=st,
                                    stop=fin,
                                )
                            for qh in range(2):
                                nc.tensor.matmul(
                                    uB[:, 512 * qh : 512 * (qh + 1)],
                                    vs[t][:, j, 65:130],
                                    eb[:, 512 * qh : 512 * (qh + 1)],
                                    start=st,
                                    stop=fin,
                                )

                        # AV(kc0) deferred to kc1: its first write to the uA/uB
                        # slots must not stall PE on the previous tile's drains
                        if kc == 0:
                            if t + 1 < NT:
                                fetch_x(t + 1)
                        else:
                            if kc == 1:
                                av(0)
                            av(kc)
                        if kc == 1 and t + 1 < NT:
                            proj_pair(t + 1)
                        if kc == 2:
                            if t >= 1:
                                epilogue(t - 1, *pend[t - 1])
                            if t == 0:
                                nc.sync.dma_start(wffpk[:], wffpk_d[:])
                    pend[t] = drain_u(t)

                # ---- tail: epilogue(5) + LN stats + FF, overlapped ----
                sums = psU.tile([1, S], F32, name="sums", tag="u")
                sumsq = psU.tile([1, S], F32, name="sumsq", tag="u")

                def stats(t):
                    st = t == 0
                    fin = t == NT - 1
                    for qh in range(2):
                        nc.tensor.matmul(
                            sums[:, 512 * qh : 512 * (qh + 1)],
                            onesr,
                            aT[t][:, 512 * qh : 512 * (qh + 1)],
                            start=st,
                            stop=fin,
                        )
                        nc.tensor.matmul(
                            sumsq[:, 512 * qh : 512 * (qh + 1)],
                            onesb[:],
                            sqs[t][:, 512 * qh : 512 * (qh + 1)],
                            start=st,
                            stop=fin,
                        )

                epilogue(NT - 1, *pend[NT - 1])
                for t in range(NT):
                    stats(t)
                mean = s3.tile([1, S], F32R, name="mean")
                nc.vector.tensor_scalar_mul(mean[:], sums[:], 1.0 / D)
                tmpa = s3.tile([1, S], F32, name="tmpa")
                nc.vector.tensor_scalar_mul(tmpa[:], sumsq[:], 1.0 / D)
                tmpb = s3.tile([1, S], F32, name="tmpb")
                nc.vector.tensor_mul(
                    tmpb[:], mean[:].bitcast(F32), mean[:].bitcast(F32)
                )
                nc.vector.scalar_tensor_tensor(
                    tmpa[:], tmpa[:], LN_EPS, tmpb[:], op0=OP.add, op1=OP.subtract
                )
                nc.scalar.sqrt(tmpb[:], tmpa[:])
                rstd = s3.tile([1, S], F32, name="rstd")
                nc.vector.reciprocal(rstd[:], tmpb[:])
                rstdB = s3.tile([128, S], F32, name="rstdB")
                nc.gpsimd.partition_broadcast(rstdB[:], rstd[:])
                meanB = s3.tile([128, S], F32R, name="meanB")
                nc.gpsimd.partition_broadcast(meanB[:], mean[:])
                if debug:
                    nc.sync.dma_start(dbg["dbg_mean"][:], mean[:].bitcast(F32))
                    nc.sync.dma_start(dbg["dbg_rstd"][:], rstd[:])

                ffs = []
                for m in range(NT):
                    ff = psS.tile([128, S], F32, name="ff", tag="s")
                    for kc in range(NT):
                        st = kc == 0
                        for qh in range(2):
                            nc.tensor.matmul(
                                ff[:, 512 * qh : 512 * (qh + 1)],
                                wffr[kc][:, 128 * m : 128 * (m + 1)],
                                aT[kc][:, 512 * qh : 512 * (qh + 1)],
                                start=st,
                                stop=False,
                            )
                    # LN fold: ff -= colsum_m * mean_j via diag(-colsum)
                    for qh in range(2):
                        nc.tensor.matmul(
                            ff[:, 512 * qh : 512 * (qh + 1)],
                            dcs_m[m],
                            meanB[:, 512 * qh : 512 * (qh + 1)],
                            start=False,
                            stop=True,
                        )
                    t1 = w3.tile([128, S], F32, name="t1", tag="t1")
                    nc.vector.tensor_mul(t1[:], ff[:], rstdB[:])
                    y = w3.tile([128, S], F32, name="y", tag="y")
                    nc.vector.scalar_tensor_tensor(
                        y[:], t1[:], gb[m], aT[m][:], op0=OP.add, op1=OP.add
                    )
                    nc.sync.dma_start(out_d[128 * m : 128 * (m + 1), :], y[:])

        if loop_n is not None:
            with tc.For_i(0, loop_n, 1) as i:
                body(i)
        else:
            body()

    nc.compile()
    return nc


def prep_inputs(x, wq, bq, wk, bk, wv, bv, wp, bp, gamma, beta, wff, bff):
    """Host-side preprocessing -> per-core input maps."""
    x = np.asarray(x, dtype=np.float32)
    wq = np.asarray(wq, np.float32)
    bq = np.asarray(bq, np.float32)
    wk = np.asarray(wk, np.float32)
    wv = np.asarray(wv, np.float32)
    wp_ = np.asarray(wp, np.float32)
    bp = np.asarray(bp, np.float32)
    bv = np.asarray(bv, np.float32)
    gamma = np.asarray(gamma, np.float32)
    beta = np.asarray(beta, np.float32)
    wff = np.asarray(wff, np.float32)
    bff = np.asarray(bff, np.float32)

    scale = np.float32(1.0 / np.sqrt(np.float32(DH)))
    wpk = np.zeros((128, WPK), np.float32)
    wpk[0:64, _WQ : _WQ + 64] = wq * scale
    wpk[64:128, _WQ + 64 : _WQ + 128] = wq * scale
    wpk[0:64, _WK : _WK + 64] = wk
    wpk[64:128, _WK + 64 : _WK + 128] = wk
    wpk[0:64, _WV : _WV + 64] = wv
    wpk[64:128, _WV + 64 : _WV + 128] = wv
    wpk[0:64, _WPA : _WPA + 64] = wp_
    wpk[64:128, _WPA + 64 : _WPA + 128] = wp_
    bpp = bv @ wp_ + bp  # v-bias folded through proj
    wpk[:, _BPA] = np.concatenate([bpp, bpp])
    wpk[:, _BQ] = np.concatenate([bq, bq]) * scale

    # channel permutation: head-major c' = h*64+dh holds original c = dh*12+h
    cp = np.arange(D)
    hh, dd = cp // 64, cp % 64
    p = dd * H + hh  # p[c'] = original channel
    wffg = wff * gamma[:, None]  # fold LN gamma into FF rows
    bffg = bff + beta @ wff  # fold LN beta through FF
    wffp = np.ascontiguousarray(wffg[p][:, p]).astype(np.float32)
    ncs = -wffp.sum(axis=0)
    dcs = np.zeros((128, NT * 128), np.float32)
    for m in range(NT):
        np.fill_diagonal(dcs[:, 128 * m : 128 * (m + 1)], ncs[128 * m : 128 * (m + 1)])
    wpk[:, _ONE] = 1.0
    wpk[:, _BFF : _BFF + NT] = bffg[p].reshape(NT, 128).T
    wffpk = np.ascontiguousarray(
        wffp.reshape(NT, 128, D).transpose(1, 0, 2).reshape(128, NT * D)
    )

    wqkvf = np.concatenate(
        [wpk[:, _WQ : _WQ + 128], wpk[:, _WK : _WK + 128],
         wpk[:, _WV : _WV + 256], wpk[:, _BQ : _BQ + 1]], axis=1
    ).copy()
    shared = {"wpk": wpk, "wffpk": wffpk, "dcs": dcs, "wqkvf": wqkvf}
    in_maps = []
    for i in range(x.shape[0]):
        m = dict(shared)
        m["xT"] = np.ascontiguousarray(x[i].T)
        in_maps.append(m)
    return in_maps, p


def postprocess(results, p):
    outs = []
    for r in results:
        yt = r["out"].T  # [S, D] head-major channels
        y = np.empty_like(yt)
        y[:, p] = yt
        outs.append(y)
    return np.stack(outs)


def kernel(**inputs) -> np.ndarray:
    if "nc" not in _CACHE:
        _CACHE["nc"] = build_nc()
    nc = _CACHE["nc"]
    in_maps, p = prep_inputs(**inputs)
    res = run_bass_kernel_spmd(nc, in_maps, list(range(8)))
    return postprocess(res.results, p)


# revision 31
# speedup vs baseline: 1.0521x; 1.0148x over previous
"""Trainium2 Bass kernel for nn_MultiHeadAttn (dense transformer block).

Contract: kernel(**inputs) takes the FULL unsharded inputs from
reference.setup_inputs() and returns the FULL output [8, 1024, 768] f32.

Sharding: data-parallel over batch N=8 -> one batch item per NeuronCore
(8 cores), no collectives.

Per-core design (channels on partitions, sequence on the free dim; host
does the boundary transposes):
  xT [768,1024] -> per 128-channel tile (2 heads):
    qT dense blockdiag (softmax scale folded), kd dense blockdiag (ONE
    drain), v natural [keys,hid] via a 256-wide moving operand (full-rate
    f32r), per-head scores via 64-partition matmuls on the dense kd/qT
    halves (tile_position rows 0/64), exp on ScalarE (PSUM drain), AV
    with a ones-column in lhsT (M=65) giving per-q exp-rowsums for free.
  Per-tile epilogue (pipelined into the next tile's score loop):
    drain uA/uB, reciprocal the rowsum rows, gpsimd-broadcast them,
    p2 = wpA'@uAs + wpB'@uBs + bpA*rowsumA + bpB*rowsumB (proj bias
    folded as rank-1 matmuls), at = p2 * recip (one DVE op), sq = at*at
    in bf16 for the LN sumsq.
  Tail: LN stats via ones-matmuls, FF on RAW at overlapping the rstd
    chain; LN algebra: normed@W = (a@W - colsum*mu)*rstd, the -colsum*mu
    folded into the FF PSUM accumulation, y = (ff*rstdB + bff) + at.
  All weights arrive in two packed blobs (2 DMAs) to minimize HWDGE
    descriptor-generation serialization at kernel start.

Channel permutation: attention output channels are produced head-major
(c' = h*64+dh) while the module interleaves (c = dh*12+h). LN is
permutation-invariant; wff rows/cols, bff, gamma, beta are permuted on
the host and the final output is unpermuted on the host. bk drops out
exactly: q.(k+bk) = q.k + (q.bk), constant per query across keys, and
softmax is shift-invariant.
"""

import numpy as np

import concourse.bacc as bacc
import concourse.mybir as mybir
import concourse.tile as tile
from concourse.bass_utils import run_bass_kernel_spmd

F32 = mybir.dt.float32
F32R = mybir.dt.float32r
BF16 = mybir.dt.bfloat16
AF = mybir.ActivationFunctionType
OP = mybir.AluOpType

S = 1024  # sequence length
D = 768  # model dim
H = 12  # heads
DH = 64  # head dim
NT = 6  # channel tiles of 128 (2 heads each)
KC = 8  # key chunks of 128
LN_EPS = 1e-5

# packed weight blob column offsets (wpk [128, WPK])
_WQ, _WK, _WV = 0, 128, 256
_WPA, _WPB = 512, 640
_BPA, _BPB = 768, 896
_BQ = 1024  # [128,1]
_NCS = 1025  # [1,768]
_ONE = 1793  # [128,1]
_BFF = 1794  # [128,6]
WPK = 1800

_CACHE = {}


def build_nc(loop_n=None, debug=False):
    """Build the single-core bass program (SPMD across 8 cores)."""
    nc = bacc.Bacc("TRN2", target_bir_lowering=False, debug=False)

    xT_d = nc.dram_tensor("xT", [D, S], F32R, kind="ExternalInput")
    wpk_d = nc.dram_tensor("wpk", [128, WPK], F32R, kind="ExternalInput")
    wffpk_d = nc.dram_tensor("wffpk", [128, NT * D], F32R, kind="ExternalInput")
    dcs_d = nc.dram_tensor("dcs", [128, NT * 128], F32R, kind="ExternalInput")
    wqkvf_d = nc.dram_tensor("wqkvf", [128, 513], F32R, kind="ExternalInput")
    out_d = nc.dram_tensor("out", [D, S], F32, kind="ExternalOutput")
    dbg = {}
    if debug:
        for nm, shp in [
            ("dbg_q", [128, S]), ("dbg_k", [128, S]), ("dbg_v", [128, 1040]),
            ("dbg_eA", [128, S]), ("dbg_eB", [128, S]),
            ("dbg_uAs", [65, S]), ("dbg_uBs", [65, S]),
            ("dbg_at", [128, S]), ("dbg_mean", [1, S]), ("dbg_rstd", [1, S]),
        ]:
            dbg[nm] = nc.dram_tensor(nm, shp, F32, kind="ExternalOutput")

    with tile.TileContext(nc) as tc:

        def body(_i=None):
            with (
                tc.tile_pool(name="const", bufs=1) as cpool,
                tc.tile_pool(name="atile", bufs=1) as apool,
                tc.tile_pool(name="psS", bufs=2, space="PSUM") as psS,
                tc.tile_pool(name="psU", bufs=2, space="PSUM") as psU,
                tc.tile_pool(name="qkv", bufs=2) as qkvpool,
                tc.tile_pool(name="xr", bufs=1) as xrpool,
                tc.tile_pool(name="ew", bufs=2) as ew,
                tc.tile_pool(name="uw", bufs=2) as uw,
                tc.tile_pool(name="w3", bufs=2) as w3,
                tc.tile_pool(name="s3", bufs=1) as s3,
            ):
                # xr(0) first: the DMA engine serializes transfers, and the
                # first q-proj only needs xr+wpk. wffpk (biggest, tail-only)
                # goes last on a separate queue.
                xr0 = xrpool.tile([128, S], F32R, name="xr", tag="xr", bufs=2)
                nc.sync.dma_start(xr0[:], xT_d[0:128, :])
                wqkvf = cpool.tile([128, 513], F32R, name="wqkvf")
                nc.sync.dma_start(wqkvf[:], wqkvf_d[:])
                wpk = cpool.tile([128, WPK], F32R, name="wpk")
                nc.sync.dma_start(wpk[:], wpk_d[:])
                wffpk2 = cpool.tile([128, NT * 128], F32R, name="wffpk2")
                nc.sync.dma_start(wffpk2[:], dcs_d[:])
                wffpk = cpool.tile([128, NT * D], F32R, name="wffpk")

                wq2r = wqkvf[:, 0:128]
                wk2r = wqkvf[:, 128:256]
                wv256r = wqkvf[:, 256:512]
                wp2r = wpk[:, _WPA : _WPA + 128]
                bp2c = wpk[:, _BPA : _BPA + 1].bitcast(F32)
                bq2 = wqkvf[:, 512:513].bitcast(F32)
                dcs_m = [
                    wffpk2[:, 128 * m : 128 * (m + 1)] for m in range(NT)
                ]
                onesr = wpk[:, _ONE : _ONE + 1]
                gb = [
                    wpk[:, _BFF + t : _BFF + t + 1].bitcast(F32)
                    for t in range(NT)
                ]
                wffr = [wffpk[:, D * t : D * (t + 1)] for t in range(NT)]
                onesb = cpool.tile([128, 1], BF16, name="onesb")
                nc.vector.tensor_copy(onesb[:], onesr[:].bitcast(F32))

                qT = [None] * NT
                kd = [None] * NT
                vs = [None] * NT
                uAp = [None] * NT
                uBp = [None] * NT
                aT = [None] * NT
                sqs = [None] * NT

                xrs = {0: xr0}

                def fetch_x(t):
                    xr = xrpool.tile([128, S], F32R, name="xr", tag="xr", bufs=2)
                    nc.sync.dma_start(xr[:], xT_d[128 * t : 128 * (t + 1), :])
                    xrs[t] = xr

                def proj_pair(t):
                    """q/k/v projections for channel tile t (v first: its
                    PSUM slots must free fastest for the score rotation)."""
                    xr = xrs[t]
                    qps = psS.tile([128, S], F32, name="qps", tag="s")
                    for qh in range(2):
                        nc.tensor.matmul(
                            qps[:, 512 * qh : 512 * (qh + 1)],
                            wq2r,
                            xr[:, 512 * qh : 512 * (qh + 1)],
                            start=True,
                            stop=True,
                        )
                    qa = qkvpool.tile([128, S], BF16, name=f"qTA{t}", tag="qa")
                    nc.vector.tensor_scalar_add(qa[0:64, :], qps[0:64, :], bq2[0:64])
                    qb = qkvpool.tile([128, S], BF16, name=f"qTB{t}", tag="qb")
                    nc.vector.tensor_scalar_add(
                        qb[64:128, :], qps[64:128, :], bq2[64:128]
                    )
                    if t < 2:  # later tiles reuse the slots; pads stay zero
                        U16 = mybir.dt.uint16
                        nc.vector.memset(qa[64:128, :].bitcast(U16), 0)
                        nc.vector.memset(qb[0:64, :].bitcast(U16), 0)
                    qT[t] = (qa, qb)
                    kps = psS.tile([128, S], F32, name="kps", tag="s")
                    for qh in range(2):
                        nc.tensor.matmul(
                            kps[:, 512 * qh : 512 * (qh + 1)],
                            wk2r,
                            xr[:, 512 * qh : 512 * (qh + 1)],
                            start=True,
                            stop=True,
                        )
                    k = qkvpool.tile([128, S], BF16, name=f"kd{t}", tag="k")
                    nc.vector.tensor_copy(k[:], kps[:])
                    kd[t] = k
                    v = qkvpool.tile([128, KC, 130], BF16, name=f"vs{t}", tag="v")
                    for g in range(2):
                        vps = psS.tile([128, 4, 256], F32, name="vps", tag="s")
                        for j in range(4):
                            c = 4 * g + j
                            nc.tensor.matmul(
                                vps[:, j, :],
                                xr[:, 128 * c : 128 * (c + 1)],
                                wv256r,
                                start=True,
                                stop=True,
                            )
                        nc.vector.tensor_copy(
                            v[:, 4 * g : 4 * (g + 1), 1:65], vps[:, :, 0:64]
                        )
                        nc.vector.tensor_copy(
                            v[:, 4 * g : 4 * (g + 1), 66:130], vps[:, :, 64:128]
                        )
                    nc.vector.memset(
                        v[:, :, 0:1].bitcast(mybir.dt.uint16), 0x3F80
                    )
                    nc.vector.memset(
                        v[:, :, 65:66].bitcast(mybir.dt.uint16), 0x3F80
                    )
                    vs[t] = v
                    if debug and t == 0:
                        pass
                        pass
                        pass
                        pass

                def drain_u(t):
                    """Drain AV accumulators; reciprocal + broadcast the
                    softmax denominators (issued at tile boundary, runs on
                    DVE/Pool off the PE critical path)."""
                    uAs = uw.tile([65, S], F32R, name="uAs", tag="uAs")
                    nc.vector.tensor_copy(uAs[:], uAp[t][:])
                    uBs = uw.tile([65, S], F32R, name="uBs", tag="uBs")
                    nc.vector.tensor_copy(uBs[:], uBp[t][:])
                    if debug and t == 0:
                        nc.sync.dma_start(dbg["dbg_uAs"][:], uAs[:].bitcast(F32))
                        nc.sync.dma_start(dbg["dbg_uBs"][:], uBs[:].bitcast(F32))
                    rcA = uw.tile([1, S], F32, name="rcA", tag="rcA", bufs=1)
                    nc.vector.reciprocal(rcA[:], uAs[0:1, :].bitcast(F32))
                    rcB = uw.tile([1, S], F32, name="rcB", tag="rcB", bufs=1)
                    nc.vector.reciprocal(rcB[:], uBs[0:1, :].bitcast(F32))
                    rb2 = uw.tile([128, S], F32, name="rb2", tag="rb2", bufs=1)
                    nc.gpsimd.partition_broadcast(rb2[0:64, :], rcA[:])
                    # partition_broadcast ignores the out AP's partition base,
                    # so broadcast B at base 0 and DMA-shift it up.
                    rbB0 = uw.tile([64, S], F32, name="rbB0", tag="rbB0", bufs=1)
                    nc.gpsimd.partition_broadcast(rbB0[:], rcB[:])
                    nc.sync.dma_start(rb2[64:128, :], rbB0[:])
                    u2 = uw.tile([128, S], F32R, name="u2", tag="u2", bufs=1)
                    nc.sync.dma_start(u2[0:64, :], uAs[1:65, :])
                    nc.sync.dma_start(u2[64:128, :], uBs[1:65, :])
                    return uAs, uBs, rb2, u2

                def epilogue(t, uAs, uBs, rb2, u2):
                    """proj + softmax-normalize for tile t (issued at kc==2
                    of tile t+1 so its PSUM slot doesn't stall scores)."""
                    p2 = psS.tile([128, S], F32, name="p2", tag="s")
                    for qh in range(2):
                        nc.tensor.matmul(
                            p2[:, 512 * qh : 512 * (qh + 1)],
                            wp2r,
                            u2[:, 512 * qh : 512 * (qh + 1)],
                            start=True,
                            stop=True,
                        )
                    a1 = uw.tile([128, S], F32, name="a1", tag="a1", bufs=1)
                    nc.vector.tensor_mul(a1[:], p2[:], rb2[:])
                    at = apool.tile([128, S], F32R, name=f"aT{t}")
                    nc.vector.tensor_scalar_add(at[:], a1[:], bp2c)
                    aT[t] = at
                    if debug and t == 0:
                        nc.sync.dma_start(dbg["dbg_at"][:], at[:].bitcast(F32))
                    sq = apool.tile([128, S], BF16, name=f"sq{t}")
                    nc.vector.tensor_mul(sq[:], at[:], at[:])
                    sqs[t] = sq

                # ---- main attention loop ----
                proj_pair(0)
                pend = {}
                for t in range(NT):
                    uA = psU.tile([65, S], F32, name="uA", tag="u")
                    uB = psU.tile([65, S], F32, name="uB", tag="u")
                    uAp[t] = uA
                    uBp[t] = uB
                    es = {}
                    for kc in range(KC):
                        sA = psS.tile([128, S], F32, name="sA", tag="s")
                        sB = psS.tile([128, S], F32, name="sB", tag="s")
                        for qh in range(2):
                            nc.tensor.matmul(
                                sA[:, 512 * qh : 512 * (qh + 1)],
                                kd[t][:, 128 * kc : 128 * (kc + 1)],
                                qT[t][0][:, 512 * qh : 512 * (qh + 1)],
                                start=True,
                                stop=True,
                            )
                        for qh in range(2):
                            nc.tensor.matmul(
                                sB[:, 512 * qh : 512 * (qh + 1)],
                                kd[t][:, 128 * kc : 128 * (kc + 1)],
                                qT[t][1][:, 512 * qh : 512 * (qh + 1)],
                                start=True,
                                stop=True,
                            )
                        eA = ew.tile([128, S], BF16, name="eA", tag="eA", bufs=3)
                        nc.scalar.activation(eA[:], sA[:], AF.Exp)
                        eB = ew.tile([128, S], BF16, name="eB", tag="eB", bufs=3)
                        nc.scalar.activation(eB[:], sB[:], AF.Exp)
                        es[kc] = (eA, eB)
                        if debug and t == 0 and kc == 0:
                            pass
                            pass

                        def av(j):
                            st = j == 0
                            fin = j == KC - 1
                            ea, eb = es[j]
                            for qh in range(2):
                                nc.tensor.matmul(
                                    uA[:, 512 * qh : 512 * (qh + 1)],
                                    vs[t][:, j, 0:65],
                                    ea[:, 512 * qh : 512 * (qh + 1)],
                                    start=st,
                                    stop=fin,
                                )
                            for qh in range(2):
                                nc.tensor.matmul(
                                    uB[:, 512 * qh : 512 * (qh + 1)],
                                    vs[t][:, j, 65:130],
                                    eb[:, 512 * qh : 512 * (qh + 1)],
                                    start=st,
                                    stop=fin,
                                )

                        # AV(kc0) deferred to kc1: its first write to the uA/uB
                        # slots must not stall PE on the previous tile's drains
                        if kc == 0:
                            if t + 1 < NT:
                                fetch_x(t + 1)
                        else:
                            if kc == 1:
                                av(0)
                            av(kc)
                        if kc == 1 and t + 1 < NT:
                            proj_pair(t + 1)
                        if kc == 2:
                            if t >= 1:
                                epilogue(t - 1, *pend[t - 1])
                            if t == 0:
                                nc.sync.dma_start(wffpk[:], wffpk_d[:])
                    pend[t] = drain_u(t)

                # ---- tail: epilogue(5) + LN stats + FF, overlapped ----
                sums = psU.tile([1, S], F32, name="sums", tag="u")
                sumsq = psU.tile([1, S], F32, name="sumsq", tag="u")

                def stats(t):
                    st = t == 0
                    fin = t == NT - 1
                    for qh in range(2):
                        nc.tensor.matmul(
                            sums[:, 512 * qh : 512 * (qh + 1)],
                            onesr,
                            aT[t][:, 512 * qh : 512 * (qh + 1)],
                            start=st,
                            stop=fin,
                        )
                        nc.tensor.matmul(
                            sumsq[:, 512 * qh : 512 * (qh + 1)],
                            onesb[:],
                            sqs[t][:, 512 * qh : 512 * (qh + 1)],
                            start=st,
                            stop=fin,
                        )

                epilogue(NT - 1, *pend[NT - 1])
                for t in range(NT):
                    stats(t)
                mean = s3.tile([1, S], F32R, name="mean")
                nc.vector.tensor_scalar_mul(mean[:], sums[:], 1.0 / D)
                tmpa = s3.tile([1, S], F32, name="tmpa")
                nc.vector.tensor_scalar_mul(tmpa[:], sumsq[:], 1.0 / D)
                tmpb = s3.tile([1, S], F32, name="tmpb")
                nc.vector.tensor_mul(
                    tmpb[:], mean[:].bitcast(F32), mean[:].bitcast(F32)
                )
                nc.vector.scalar_tensor_tensor(
                    tmpa[:], tmpa[:], LN_EPS, tmpb[:], op0=OP.add, op1=OP.subtract
                )
                nc.scalar.sqrt(tmpb[:], tmpa[:])
                rstd = s3.tile([1, S], F32, name="rstd")
                nc.vector.reciprocal(rstd[:], tmpb[:])
                rstdB = s3.tile([128, S], F32, name="rstdB")
                nc.gpsimd.partition_broadcast(rstdB[:], rstd[:])
                meanB = s3.tile([128, S], F32R, name="meanB")
                nc.gpsimd.partition_broadcast(meanB[:], mean[:])
                if debug:
                    nc.sync.dma_start(dbg["dbg_mean"][:], mean[:].bitcast(F32))
                    nc.sync.dma_start(dbg["dbg_rstd"][:], rstd[:])

                ffs = []
                for m in range(NT):
                    ff = psS.tile([128, S], F32, name="ff", tag="s")
                    for kc in range(NT):
                        st = kc == 0
                        for qh in range(2):
                            nc.tensor.matmul(
                                ff[:, 512 * qh : 512 * (qh + 1)],
                                wffr[kc][:, 128 * m : 128 * (m + 1)],
                                aT[kc][:, 512 * qh : 512 * (qh + 1)],
                                start=st,
                                stop=False,
                            )
                    # LN fold: ff -= colsum_m * mean_j via diag(-colsum)
                    for qh in range(2):
                        nc.tensor.matmul(
                            ff[:, 512 * qh : 512 * (qh + 1)],
                            dcs_m[m],
                            meanB[:, 512 * qh : 512 * (qh + 1)],
                            start=False,
                            stop=True,
                        )
                    t1 = w3.tile([128, S], F32, name="t1", tag="t1")
                    nc.vector.tensor_mul(t1[:], ff[:], rstdB[:])
                    y = w3.tile([128, S], F32, name="y", tag="y")
                    nc.vector.scalar_tensor_tensor(
                        y[:], t1[:], gb[m], aT[m][:], op0=OP.add, op1=OP.add
                    )
                    nc.sync.dma_start(out_d[128 * m : 128 * (m + 1), :], y[:])

        if loop_n is not None:
            with tc.For_i(0, loop_n, 1) as i:
                body(i)
        else:
            body()

    nc.compile()
    return nc


def prep_inputs(x, wq, bq, wk, bk, wv, bv, wp, bp, gamma, beta, wff, bff):
    """Host-side preprocessing -> per-core input maps."""
    x = np.asarray(x, dtype=np.float32)
    wq = np.asarray(wq, np.float32)
    bq = np.asarray(bq, np.float32)
    wk = np.asarray(wk, np.float32)
    wv = np.asarray(wv, np.float32)
    wp_ = np.asarray(wp, np.float32)
    bp = np.asarray(bp, np.float32)
    bv = np.asarray(bv, np.float32)
    gamma = np.asarray(gamma, np.float32)
    beta = np.asarray(beta, np.float32)
    wff = np.asarray(wff, np.float32)
    bff = np.asarray(bff, np.float32)

    scale = np.float32(1.0 / np.sqrt(np.float32(DH)))
    wpk = np.zeros((128, WPK), np.float32)
    wpk[0:64, _WQ : _WQ + 64] = wq * scale
    wpk[64:128, _WQ + 64 : _WQ + 128] = wq * scale
    wpk[0:64, _WK : _WK + 64] = wk
    wpk[64:128, _WK + 64 : _WK + 128] = wk
    wpk[0:64, _WV : _WV + 64] = wv
    wpk[64:128, _WV + 64 : _WV + 128] = wv
    wpk[0:64, _WPA : _WPA + 64] = wp_
    wpk[64:128, _WPA + 64 : _WPA + 128] = wp_
    bpp = bv @ wp_ + bp  # v-bias folded through proj
    wpk[:, _BPA] = np.concatenate([bpp, bpp])
    wpk[:, _BQ] = np.concatenate([bq, bq]) * scale

    # channel permutation: head-major c' = h*64+dh holds original c = dh*12+h
    cp = np.arange(D)
    hh, dd = cp // 64, cp % 64
    p = dd * H + hh  # p[c'] = original channel
    wffg = wff * gamma[:, None]  # fold LN gamma into FF rows
    bffg = bff + beta @ wff  # fold LN beta through FF
    wffp = np.ascontiguousarray(wffg[p][:, p]).astype(np.float32)
    ncs = -wffp.sum(axis=0)
    dcs = np.zeros((128, NT * 128), np.float32)
    for m in range(NT):
        np.fill_diagonal(dcs[:, 128 * m : 128 * (m + 1)], ncs[128 * m : 128 * (m + 1)])
    wpk[:, _ONE] = 1.0
    wpk[:, _BFF : _BFF + NT] = bffg[p].reshape(NT, 128).T
    wffpk = np.ascontiguousarray(
        wffp.reshape(NT, 128, D).transpose(1, 0, 2).reshape(128, NT * D)
    )

    wqkvf = np.concatenate(
        [wpk[:, _WQ : _WQ + 128], wpk[:, _WK : _WK + 128],
         wpk[:, _WV : _WV + 256], wpk[:, _BQ : _BQ + 1]], axis=1
    ).copy()
    shared = {"wpk": wpk, "wffpk": wffpk, "dcs": dcs, "wqkvf": wqkvf}
    in_maps = []
    for i in range(x.shape[0]):
        m = dict(shared)
        m["xT"] = np.ascontiguousarray(x[i].T)
        in_maps.append(m)
    return in_maps, p


def postprocess(results, p):
    outs = []
    for r in results:
        yt = r["out"].T  # [S, D] head-major channels
        y = np.empty_like(yt)
        y[:, p] = yt
        outs.append(y)
    return np.stack(outs)


def kernel(**inputs) -> np.ndarray:
    if "nc" not in _CACHE:
        _CACHE["nc"] = build_nc()
    nc = _CACHE["nc"]
    in_maps, p = prep_inputs(**inputs)
    res = run_bass_kernel_spmd(nc, in_maps, list(range(8)))
    return postprocess(res.results, p)
